# revision 3
# baseline (speedup 1.0000x reference)
"""Mixtral-style MoE block (T=2048, H=1024, F=2048, E=8, top-2) on 8 trn2
NeuronCores.

Expert-parallel with host-side top-2 dispatch: the (tiny) router runs on the
host in fp32, exactly mirroring the reference math; each core receives only
the tokens routed to its expert (capacity C = max expert load), in bf16,
plus a second copy of the tokens pre-scaled by the renormalized top-2
combine weight (so the combine scaling rides the linear w3 branch for free).
Each core computes out_e = (silu(x w1) * (x_cw w3)) @ w2 in bf16 with fp32
PSUM accumulation and writes its [H, C] partial; the host scatter-adds the
two partials per token.  No device collectives; weights stream once (bf16,
12.6 MB/core) and the kernel is PE-bound at ~78 TF/s bf16.
"""
import numpy as np

try:
    import concourse  # noqa: F401
except ImportError:  # pragma: no cover
    import sys
    sys.path.insert(0, "/opt/trn_rl_repo")

import ml_dtypes
from concourse import mybir, bacc
import concourse.tile as tile
from concourse.bass_utils import run_bass_kernel_spmd

T, H, F, E, TOP_K = 2048, 1024, 2048, 8, 2
P = 128
KH = H // P    # 8  k-tiles over H (mm1/mm3 contraction)
KF = F // P    # 16 k-tiles over F (mm2 contraction)
MF = F // P    # 16 m-tiles over F (mm1/mm3 output partitions)
MH = H // P    # 8  m-tiles over H (mm2 output partitions)
F32 = mybir.dt.float32
BF16 = mybir.dt.bfloat16
BF16NP = ml_dtypes.bfloat16
PSUM = "PSUM"

_NC_CACHE = {}


def _chunks(C):
    """Token-stream chunks of <=512 columns (PSUM bank limit)."""
    cs, s = [], 0
    while s < C:
        cs.append((s, min(s + 512, C)))
        s = min(s + 512, C)
    return cs


def build(C):
    nc = bacc.Bacc("TRN2", target_bir_lowering=False, debug=False,
                   num_devices=E)
    xt = nc.dram_tensor("xt", [KH, P, C], BF16, kind="ExternalInput")
    xts = nc.dram_tensor("xts", [KH, P, C], BF16, kind="ExternalInput")
    w1t = nc.dram_tensor("w1t", [MF, P, KH, P], BF16, kind="ExternalInput")
    w3t = nc.dram_tensor("w3t", [MF, P, KH, P], BF16, kind="ExternalInput")
    w2t = nc.dram_tensor("w2t", [MH, P, KF, P], BF16, kind="ExternalInput")
    outT = nc.dram_tensor("outT", [H, C], F32, kind="ExternalOutput")

    CH = _chunks(C)
    psbufs = 2 if len(CH) <= 2 else 1
    with tile.TileContext(nc) as tc:
        with (
            tc.tile_pool(name="big", bufs=1) as big,
            tc.tile_pool(name="wpool", bufs=3) as wpool,
            tc.tile_pool(name="w2pool", bufs=3) as w2pool,
            tc.tile_pool(name="evac", bufs=4) as evac,
        ):
            xt_s = big.tile([P, KH, C], BF16)
            nc.sync.dma_start(out=xt_s[:],
                              in_=xt.ap().rearrange("k p c -> p k c"))
            xts_s = big.tile([P, KH, C], BF16)
            nc.sync.dma_start(out=xts_s[:],
                              in_=xts.ap().rearrange("k p c -> p k c"))
            inter = big.tile([P, KF, C], BF16)

            w1v = w1t.ap().rearrange("m p k f -> p m k f")
            w3v = w3t.ap().rearrange("m p k f -> p m k f")
            w2v = w2t.ap().rearrange("m p k h -> p m k h")

            # Phase A: interT[f, t] = silu(w1.T x) * (w3.T x_cw)
            with tc.tile_pool(name="psA", bufs=psbufs, space=PSUM) as psA:
                for m in range(MF):
                    w1m = wpool.tile([P, KH, P], BF16, tag="w1m")
                    nc.gpsimd.dma_start(out=w1m[:], in_=w1v[:, m])
                    w3m = wpool.tile([P, KH, P], BF16, tag="w3m")
                    nc.gpsimd.dma_start(out=w3m[:], in_=w3v[:, m])
                    ps1 = [psA.tile([P, e - s], F32, tag=f"ps1_{i}", name=f"ps1_{i}")
                           for i, (s, e) in enumerate(CH)]
                    ps3 = [psA.tile([P, e - s], F32, tag=f"ps3_{i}", name=f"ps3_{i}")
                           for i, (s, e) in enumerate(CH)]
                    for k in range(KH):
                        for i, (s, e) in enumerate(CH):
                            nc.tensor.matmul(ps1[i][:], lhsT=w1m[:, k, :],
                                             rhs=xt_s[:, k, s:e],
                                             start=(k == 0), stop=(k == KH - 1))
                    for k in range(KH):
                        for i, (s, e) in enumerate(CH):
                            nc.tensor.matmul(ps3[i][:], lhsT=w3m[:, k, :],
                                             rhs=xts_s[:, k, s:e],
                                             start=(k == 0), stop=(k == KH - 1))
                    for i, (s, e) in enumerate(CH):
                        sil = evac.tile([P, e - s], F32, tag=f"sil_{i}")
                        nc.scalar.activation(sil[:], ps1[i][:],
                                             mybir.ActivationFunctionType.Silu)
                        nc.vector.tensor_tensor(inter[:, m, s:e], sil[:],
                                                ps3[i][:],
                                                op=mybir.AluOpType.mult)

            # Phase B: outT[h, t] = w2.T @ inter
            with tc.tile_pool(name="psB", bufs=psbufs, space=PSUM) as psB:
                for m in range(MH):
                    w2m = w2pool.tile([P, KF, P], BF16, tag="w2m")
                    nc.gpsimd.dma_start(out=w2m[:], in_=w2v[:, m])
                    ps = [psB.tile([P, e - s], F32, tag=f"ps_{i}", name=f"ps_{i}")
                          for i, (s, e) in enumerate(CH)]
                    for k in range(KF):
                        for i, (s, e) in enumerate(CH):
                            nc.tensor.matmul(ps[i][:], lhsT=w2m[:, k, :],
                                             rhs=inter[:, k, s:e],
                                             start=(k == 0), stop=(k == KF - 1))
                    o = evac.tile([P, C], F32, tag="o")
                    for i, (s, e) in enumerate(CH):
                        nc.vector.tensor_copy(o[:, s:e], ps[i][:])
                    nc.sync.dma_start(out=outT.ap()[m * P:(m + 1) * P, :],
                                      in_=o[:])
    nc.compile()
    return nc


def _route(hidden_states, gate_w):
    """Host router mirroring the reference fp32 math exactly."""
    logits = (hidden_states.astype(np.float32) @
              gate_w.astype(np.float32)).astype(np.float32)
    mx = logits.max(axis=-1, keepdims=True)
    p = np.exp(logits - mx)
    p /= p.sum(axis=-1, keepdims=True)
    idx = np.argsort(-p, axis=-1, kind="stable")[:, :TOP_K]
    tw = np.take_along_axis(p, idx, axis=-1)
    tw = tw / tw.sum(axis=-1, keepdims=True)
    return idx, tw


def _plan(hidden_states, gate_w):
    idx, tw = _route(hidden_states, gate_w)
    toks, cws = [], []
    for e in range(E):
        mask = idx == e
        tok = np.nonzero(mask.any(axis=1))[0]
        toks.append(tok)
        cws.append((tw * mask).sum(axis=1)[tok].astype(np.float32))
    cap = max(1, max(len(t) for t in toks))
    C = (cap + 7) // 8 * 8
    return toks, cws, C


def make_in_maps(hidden_states, gate_w, w1, w2, w3):
    x = np.asarray(hidden_states, dtype=np.float32)
    toks, cws, C = _plan(x, np.asarray(gate_w, dtype=np.float32))
    in_maps = []
    for e in range(E):
        tok, cw = toks[e], cws[e]
        n = len(tok)
        xe = x[tok]                                   # [n, H] fp32
        xt = np.zeros((KH, P, C), dtype=BF16NP)
        xt[:, :, :n] = xe.T.reshape(KH, P, n).astype(BF16NP)
        xts = np.zeros((KH, P, C), dtype=BF16NP)
        xts[:, :, :n] = (xe * cw[:, None]).T.reshape(KH, P, n).astype(BF16NP)
        w1e = np.asarray(w1[e], dtype=np.float32)
        w3e = np.asarray(w3[e], dtype=np.float32)
        w2e = np.asarray(w2[e], dtype=np.float32)
        w1tt = np.ascontiguousarray(
            w1e.reshape(KH, P, MF, P).transpose(2, 1, 0, 3)).astype(BF16NP)
        w3tt = np.ascontiguousarray(
            w3e.reshape(KH, P, MF, P).transpose(2, 1, 0, 3)).astype(BF16NP)
        w2tt = np.ascontiguousarray(
            w2e.reshape(KF, P, MH, P).transpose(2, 1, 0, 3)).astype(BF16NP)
        in_maps.append({"xt": xt, "xts": xts,
                        "w1t": w1tt, "w3t": w3tt, "w2t": w2tt})
    return in_maps


def kernel(hidden_states, gate_w, w1, w2, w3):
    x = np.asarray(hidden_states, dtype=np.float32)
    gw = np.asarray(gate_w, dtype=np.float32)
    toks, cws, C = _plan(x, gw)
    if _NC_CACHE.get("C") != C:
        _NC_CACHE["nc"] = build(C)
        _NC_CACHE["C"] = C
    nc = _NC_CACHE["nc"]

    in_maps = make_in_maps(x, gw, w1, w2, w3)
    res = run_bass_kernel_spmd(nc, in_maps, core_ids=list(range(E)),
                               trace=False)
    out = np.zeros((T, H), dtype=np.float32)
    for e in range(E):
        tok = toks[e]
        if len(tok):
            out[tok] += res.results[e]["outT"][:, :len(tok)].T
    return out


# revision 4
# speedup vs baseline: 1.0837x; 1.0837x over previous
"""Mixtral-style MoE block (T=2048, H=1024, F=2048, E=8, top-2) on 8 trn2
NeuronCores.

Expert-parallel with host-side top-2 dispatch: the (tiny) router runs on the
host in fp32, exactly mirroring the reference math; each core receives only
the tokens routed to its expert (capacity C = max expert load), in bf16,
plus a second copy of the tokens pre-scaled by the renormalized top-2
combine weight (so the combine scaling rides the linear w3 branch for free).
Each core computes out_e = (silu(x w1) * (x_cw w3)) @ w2 in bf16 with fp32
PSUM accumulation and writes its [H, C] partial; the host scatter-adds the
two partials per token.  No device collectives; weights stream once (bf16,
12.6 MB/core, hardware-DGE contiguous chunks) and the kernel is PE-bound
at ~78 TF/s bf16.
"""
import numpy as np

try:
    import concourse  # noqa: F401
except ImportError:  # pragma: no cover
    import sys
    sys.path.insert(0, "/opt/trn_rl_repo")

import ml_dtypes
from concourse import mybir, bacc
import concourse.tile as tile
from concourse.bass_utils import run_bass_kernel_spmd

T, H, F, E, TOP_K = 2048, 1024, 2048, 8, 2
P = 128
KH = H // P    # 8  k-tiles over H (mm1/mm3 contraction)
KF = F // P    # 16 k-tiles over F (mm2 contraction)
MF = F // P    # 16 m-tiles over F (mm1/mm3 output partitions)
MH = H // P    # 8  m-tiles over H (mm2 output partitions)
WG = 2         # m-tiles per w1/w3 DMA chunk (512 KB each)
WG2 = 2        # m-tiles per w2 DMA chunk (1 MB each)
F32 = mybir.dt.float32
BF16 = mybir.dt.bfloat16
BF16NP = ml_dtypes.bfloat16
PSUM = "PSUM"

_NC_CACHE = {}


def _chunks(C):
    """Equal token-stream chunks of <=512 columns (PSUM bank limit)."""
    n = (C + 511) // 512
    base = C // n // 4 * 4
    cs, s = [], 0
    for i in range(n):
        e = C if i == n - 1 else s + base
        cs.append((s, e))
        s = e
    return cs


def build(C):
    nc = bacc.Bacc("TRN2", target_bir_lowering=False, debug=False,
                   num_devices=E)
    # host pre-tiles everything so each DMA is contiguous per partition
    xt = nc.dram_tensor("xt", [P, KH, C], BF16, kind="ExternalInput")
    xts = nc.dram_tensor("xts", [P, KH, C], BF16, kind="ExternalInput")
    NW = MF // WG
    NW2 = MH // WG2
    w1t = nc.dram_tensor("w1t", [NW, P, WG, KH, P], BF16,
                         kind="ExternalInput")
    w3t = nc.dram_tensor("w3t", [NW, P, WG, KH, P], BF16,
                         kind="ExternalInput")
    w2t = nc.dram_tensor("w2t", [NW2, P, WG2, KF, P], BF16,
                         kind="ExternalInput")
    outT = nc.dram_tensor("outT", [H, C], F32, kind="ExternalOutput")

    CH = _chunks(C)
    psbufs = 2 if len(CH) <= 2 else 1
    with tile.TileContext(nc) as tc:
        with (
            tc.tile_pool(name="big", bufs=1) as big,
            tc.tile_pool(name="evac", bufs=4) as evac,
        ):
            # tokens on the scalar HWDGE queue
            xt_s = big.tile([P, KH, C], BF16)
            nc.scalar.dma_start(out=xt_s[:], in_=xt.ap())
            xts_s = big.tile([P, KH, C], BF16)
            nc.scalar.dma_start(out=xts_s[:], in_=xts.ap())
            inter = big.tile([P, KF, C], BF16)

            # all weights SBUF-resident, chunked contiguous DMAs on the
            # sync HWDGE queue, in PE consumption order
            w1c = [big.tile([P, WG, KH, P], BF16, name=f"w1c{j}")
                   for j in range(NW)]
            w3c = [big.tile([P, WG, KH, P], BF16, name=f"w3c{j}")
                   for j in range(NW)]
            w2c = [big.tile([P, WG2, KF, P], BF16, name=f"w2c{j}")
                   for j in range(NW2)]
            for j in range(NW):
                nc.sync.dma_start(out=w1c[j][:], in_=w1t.ap()[j])
                nc.sync.dma_start(out=w3c[j][:], in_=w3t.ap()[j])
            for j in range(NW2):
                nc.sync.dma_start(out=w2c[j][:], in_=w2t.ap()[j])

            # Phase A: interT[f, t] = silu(w1.T x) * (w3.T x_cw)
            with tc.tile_pool(name="psA", bufs=psbufs, space=PSUM) as psA:
                for m in range(MF):
                    w1m = w1c[m // WG][:, m % WG]
                    w3m = w3c[m // WG][:, m % WG]
                    ps1 = [psA.tile([P, e - s], F32, tag=f"ps1_{i}",
                                    name=f"ps1_{i}")
                           for i, (s, e) in enumerate(CH)]
                    ps3 = [psA.tile([P, e - s], F32, tag=f"ps3_{i}",
                                    name=f"ps3_{i}")
                           for i, (s, e) in enumerate(CH)]
                    for k in range(KH):
                        for i, (s, e) in enumerate(CH):
                            nc.tensor.matmul(ps1[i][:], lhsT=w1m[:, k, :],
                                             rhs=xt_s[:, k, s:e],
                                             start=(k == 0), stop=(k == KH - 1))
                    for k in range(KH):
                        for i, (s, e) in enumerate(CH):
                            nc.tensor.matmul(ps3[i][:], lhsT=w3m[:, k, :],
                                             rhs=xts_s[:, k, s:e],
                                             start=(k == 0), stop=(k == KH - 1))
                    for i, (s, e) in enumerate(CH):
                        sil = evac.tile([P, e - s], F32, tag=f"sil_{i}")
                        nc.scalar.activation(sil[:], ps1[i][:],
                                             mybir.ActivationFunctionType.Silu)
                        nc.vector.tensor_tensor(inter[:, m, s:e], sil[:],
                                                ps3[i][:],
                                                op=mybir.AluOpType.mult)

            # Phase B: outT[h, t] = w2.T @ inter
            with tc.tile_pool(name="psB", bufs=psbufs, space=PSUM) as psB:
                for m in range(MH):
                    w2m = w2c[m // WG2][:, m % WG2]
                    ps = [psB.tile([P, e - s], F32, tag=f"ps_{i}",
                                   name=f"ps_{i}")
                          for i, (s, e) in enumerate(CH)]
                    for k in range(KF):
                        for i, (s, e) in enumerate(CH):
                            nc.tensor.matmul(ps[i][:], lhsT=w2m[:, k, :],
                                             rhs=inter[:, k, s:e],
                                             start=(k == 0), stop=(k == KF - 1))
                    o = evac.tile([P, C], F32, tag="o")
                    for i, (s, e) in enumerate(CH):
                        nc.vector.tensor_copy(o[:, s:e], ps[i][:])
                    nc.scalar.dma_start(out=outT.ap()[m * P:(m + 1) * P, :],
                                        in_=o[:])
    nc.compile()
    return nc


def _route(hidden_states, gate_w):
    """Host router mirroring the reference fp32 math exactly."""
    logits = (hidden_states.astype(np.float32) @
              gate_w.astype(np.float32)).astype(np.float32)
    mx = logits.max(axis=-1, keepdims=True)
    p = np.exp(logits - mx)
    p /= p.sum(axis=-1, keepdims=True)
    idx = np.argsort(-p, axis=-1, kind="stable")[:, :TOP_K]
    tw = np.take_along_axis(p, idx, axis=-1)
    tw = tw / tw.sum(axis=-1, keepdims=True)
    return idx, tw


def _plan(hidden_states, gate_w):
    idx, tw = _route(hidden_states, gate_w)
    toks, cws = [], []
    for e in range(E):
        mask = idx == e
        tok = np.nonzero(mask.any(axis=1))[0]
        toks.append(tok)
        cws.append((tw * mask).sum(axis=1)[tok].astype(np.float32))
    cap = max(1, max(len(t) for t in toks))
    C = (cap + 7) // 8 * 8
    return toks, cws, C


def make_in_maps(hidden_states, gate_w, w1, w2, w3):
    x = np.asarray(hidden_states, dtype=np.float32)
    toks, cws, C = _plan(x, np.asarray(gate_w, dtype=np.float32))
    NW = MF // WG
    NW2 = MH // WG2
    in_maps = []
    for e in range(E):
        tok, cw = toks[e], cws[e]
        n = len(tok)
        xe = x[tok]                                   # [n, H] fp32
        xt = np.zeros((P, KH, C), dtype=BF16NP)
        xt[:, :, :n] = xe.T.reshape(KH, P, n).transpose(1, 0, 2).astype(BF16NP)
        xts = np.zeros((P, KH, C), dtype=BF16NP)
        xts[:, :, :n] = (xe * cw[:, None]).T.reshape(KH, P, n)\
            .transpose(1, 0, 2).astype(BF16NP)
        w1e = np.asarray(w1[e], dtype=np.float32)
        w3e = np.asarray(w3[e], dtype=np.float32)
        w2e = np.asarray(w2[e], dtype=np.float32)
        # [H, F] -> [NW, P, WG, KH, P] so each chunk DMA is contiguous
        w1tt = np.ascontiguousarray(
            w1e.reshape(KH, P, NW, WG, P).transpose(2, 1, 3, 0, 4)
        ).astype(BF16NP)
        w3tt = np.ascontiguousarray(
            w3e.reshape(KH, P, NW, WG, P).transpose(2, 1, 3, 0, 4)
        ).astype(BF16NP)
        w2tt = np.ascontiguousarray(
            w2e.reshape(KF, P, NW2, WG2, P).transpose(2, 1, 3, 0, 4)
        ).astype(BF16NP)
        in_maps.append({"xt": xt, "xts": xts,
                        "w1t": w1tt, "w3t": w3tt, "w2t": w2tt})
    return in_maps


def kernel(hidden_states, gate_w, w1, w2, w3):
    x = np.asarray(hidden_states, dtype=np.float32)
    gw = np.asarray(gate_w, dtype=np.float32)
    toks, cws, C = _plan(x, gw)
    if _NC_CACHE.get("C") != C:
        _NC_CACHE["nc"] = build(C)
        _NC_CACHE["C"] = C
    nc = _NC_CACHE["nc"]

    in_maps = make_in_maps(x, gw, w1, w2, w3)
    res = run_bass_kernel_spmd(nc, in_maps, core_ids=list(range(E)),
                               trace=False)
    out = np.zeros((T, H), dtype=np.float32)
    for e in range(E):
        tok = toks[e]
        if len(tok):
            out[tok] += res.results[e]["outT"][:, :len(tok)].T
    return out


# revision 5
# speedup vs baseline: 1.0860x; 1.0022x over previous
"""Mixtral-style MoE block (T=2048, H=1024, F=2048, E=8, top-2) on 8 trn2
NeuronCores.

Expert-parallel with host-side top-2 dispatch: the (tiny) router runs on the
host in fp32, exactly mirroring the reference math; each core receives only
the tokens routed to its expert (capacity C = max expert load), in bf16,
plus a second copy of the tokens pre-scaled by the renormalized top-2
combine weight (so the combine scaling rides the linear w3 branch for free).
Each core computes out_e = (silu(x w1) * (x_cw w3)) @ w2 in bf16 with fp32
PSUM accumulation and writes its [H, C] partial in bf16; the host
scatter-adds the two partials per token.  No device collectives; weights
stream once (bf16, 12.6 MB/core, hardware-DGE contiguous chunks) and the
kernel is PE-bound at ~78 TF/s bf16.
"""
import numpy as np

try:
    import concourse  # noqa: F401
except ImportError:  # pragma: no cover
    import sys
    sys.path.insert(0, "/opt/trn_rl_repo")

import ml_dtypes
from concourse import mybir, bacc
import concourse.tile as tile
from concourse.bass_utils import run_bass_kernel_spmd

T, H, F, E, TOP_K = 2048, 1024, 2048, 8, 2
P = 128
KH = H // P    # 8  k-tiles over H (mm1/mm3 contraction)
KF = F // P    # 16 k-tiles over F (mm2 contraction)
MF = F // P    # 16 m-tiles over F (mm1/mm3 output partitions)
MH = H // P    # 8  m-tiles over H (mm2 output partitions)
WG2 = 2        # m-tiles per w2 DMA chunk (1 MB each)
NW2 = MH // WG2
F32 = mybir.dt.float32
BF16 = mybir.dt.bfloat16
BF16NP = ml_dtypes.bfloat16
PSUM = "PSUM"

_NC_CACHE = {}


def _chunks(C):
    """Equal token-stream chunks of <=512 columns (PSUM bank limit)."""
    n = (C + 511) // 512
    base = C // n // 4 * 4
    cs, s = [], 0
    for i in range(n):
        e = C if i == n - 1 else s + base
        cs.append((s, e))
        s = e
    return cs


def build(C):
    nc = bacc.Bacc("TRN2", target_bir_lowering=False, debug=False,
                   num_devices=E)
    # host pre-tiles everything so each DMA is contiguous per partition
    xt = nc.dram_tensor("xt", [KH, P, C], BF16, kind="ExternalInput")
    xts = nc.dram_tensor("xts", [KH, P, C], BF16, kind="ExternalInput")
    w1t = nc.dram_tensor("w1t", [MF, P, KH, P], BF16, kind="ExternalInput")
    w3t = nc.dram_tensor("w3t", [MF, P, KH, P], BF16, kind="ExternalInput")
    w2t = nc.dram_tensor("w2t", [NW2, P, WG2, KF, P], BF16,
                         kind="ExternalInput")
    outT = nc.dram_tensor("outT", [H, C], BF16, kind="ExternalOutput")

    CH = _chunks(C)
    psbufs_a = 2 if len(CH) <= 2 else 1
    psbufs_b = 4 if len(CH) <= 2 else 2
    with tile.TileContext(nc) as tc:
        with (
            tc.tile_pool(name="big", bufs=1) as big,
            tc.tile_pool(name="evac", bufs=4) as evac,
        ):
            # PE warmup so HAM un-throttles before real work arrives
            jt = big.tile([P, 512], BF16)
            nc.gpsimd.memset(jt[:], 0.0)
            with tc.tile_pool(name="warm", bufs=1, space=PSUM) as wps:
                jp = wps.tile([P, 512], F32)
                for _ in range(8):
                    nc.tensor.matmul(jp[:], lhsT=jt[:, :P], rhs=jt[:],
                                     start=True, stop=True)

            # tokens: xt per-k tiles on the scalar HWDGE ring,
            # xts on the gpsimd (SWDGE) ring so they load in parallel
            xt_k = []
            for k in range(KH):
                t = big.tile([P, C], BF16, name=f"xt{k}")
                nc.scalar.dma_start(out=t[:], in_=xt.ap()[k])
                xt_k.append(t)
            xts_k = []
            for k in range(KH):
                t = big.tile([P, C], BF16, name=f"xts{k}")
                nc.gpsimd.dma_start(out=t[:], in_=xts.ap()[k])
                xts_k.append(t)
            inter = big.tile([P, KF, C], BF16)

            # all weights SBUF-resident, contiguous DMAs on the sync
            # HWDGE ring, in PE consumption order
            w1c, w3c = [], []
            for m in range(MF):
                t1 = big.tile([P, KH, P], BF16, name=f"w1c{m}")
                nc.sync.dma_start(out=t1[:], in_=w1t.ap()[m])
                w1c.append(t1)
                t3 = big.tile([P, KH, P], BF16, name=f"w3c{m}")
                nc.sync.dma_start(out=t3[:], in_=w3t.ap()[m])
                w3c.append(t3)
            w2c = []
            for j in range(NW2):
                t = big.tile([P, WG2, KF, P], BF16, name=f"w2c{j}")
                nc.sync.dma_start(out=t[:], in_=w2t.ap()[j])
                w2c.append(t)

            # Phase A: interT[f, t] = silu(w1.T x) * (w3.T x_cw)
            with tc.tile_pool(name="psA", bufs=psbufs_a, space=PSUM) as psA:
                for m in range(MF):
                    ps1 = [psA.tile([P, e - s], F32, tag=f"ps1_{i}",
                                    name=f"ps1_{i}")
                           for i, (s, e) in enumerate(CH)]
                    ps3 = [psA.tile([P, e - s], F32, tag=f"ps3_{i}",
                                    name=f"ps3_{i}")
                           for i, (s, e) in enumerate(CH)]
                    for i, (s, e) in enumerate(CH):
                        for k in range(KH):
                            nc.tensor.matmul(ps1[i][:], lhsT=w1c[m][:, k, :],
                                             rhs=xt_k[k][:, s:e],
                                             start=(k == 0), stop=(k == KH - 1))
                    for i, (s, e) in enumerate(CH):
                        for k in range(KH):
                            nc.tensor.matmul(ps3[i][:], lhsT=w3c[m][:, k, :],
                                             rhs=xts_k[k][:, s:e],
                                             start=(k == 0), stop=(k == KH - 1))
                    for i, (s, e) in enumerate(CH):
                        sil = evac.tile([P, e - s], F32, tag=f"sil_{i}")
                        nc.scalar.activation(sil[:], ps1[i][:],
                                             mybir.ActivationFunctionType.Silu)
                        nc.vector.tensor_tensor(inter[:, m, s:e], sil[:],
                                                ps3[i][:],
                                                op=mybir.AluOpType.mult)

            # Phase B: outT[h, t] = w2.T @ inter
            with tc.tile_pool(name="psB", bufs=psbufs_b, space=PSUM) as psB:
                for m in range(MH):
                    w2m = w2c[m // WG2][:, m % WG2]
                    for i, (s, e) in enumerate(CH):
                        ps = psB.tile([P, e - s], F32, tag=f"ps_{i}",
                                      name=f"ps_{i}")
                        for k in range(KF):
                            nc.tensor.matmul(ps[:], lhsT=w2m[:, k, :],
                                             rhs=inter[:, k, s:e],
                                             start=(k == 0), stop=(k == KF - 1))
                        o = evac.tile([P, e - s], BF16, tag=f"o_{i}")
                        nc.vector.tensor_copy(o[:], ps[:])
                        nc.scalar.dma_start(
                            out=outT.ap()[m * P:(m + 1) * P, s:e], in_=o[:])
    nc.compile()
    return nc


def _route(hidden_states, gate_w):
    """Host router mirroring the reference fp32 math exactly."""
    logits = (hidden_states.astype(np.float32) @
              gate_w.astype(np.float32)).astype(np.float32)
    mx = logits.max(axis=-1, keepdims=True)
    p = np.exp(logits - mx)
    p /= p.sum(axis=-1, keepdims=True)
    idx = np.argsort(-p, axis=-1, kind="stable")[:, :TOP_K]
    tw = np.take_along_axis(p, idx, axis=-1)
    tw = tw / tw.sum(axis=-1, keepdims=True)
    return idx, tw


def _plan(hidden_states, gate_w):
    idx, tw = _route(hidden_states, gate_w)
    toks, cws = [], []
    for e in range(E):
        mask = idx == e
        tok = np.nonzero(mask.any(axis=1))[0]
        toks.append(tok)
        cws.append((tw * mask).sum(axis=1)[tok].astype(np.float32))
    cap = max(1, max(len(t) for t in toks))
    C = (cap + 7) // 8 * 8
    return toks, cws, C


def make_in_maps(hidden_states, gate_w, w1, w2, w3):
    x = np.asarray(hidden_states, dtype=np.float32)
    toks, cws, C = _plan(x, np.asarray(gate_w, dtype=np.float32))
    in_maps = []
    for e in range(E):
        tok, cw = toks[e], cws[e]
        n = len(tok)
        xe = x[tok]                                   # [n, H] fp32
        xt = np.zeros((KH, P, C), dtype=BF16NP)
        xt[:, :, :n] = xe.T.reshape(KH, P, n).astype(BF16NP)
        xts = np.zeros((KH, P, C), dtype=BF16NP)
        xts[:, :, :n] = (xe * cw[:, None]).T.reshape(KH, P, n).astype(BF16NP)
        w1e = np.asarray(w1[e], dtype=np.float32)
        w3e = np.asarray(w3[e], dtype=np.float32)
        w2e = np.asarray(w2[e], dtype=np.float32)
        # [H, F] -> [MF, P, KH, P] so each m-tile DMA is contiguous
        w1tt = np.ascontiguousarray(
            w1e.reshape(KH, P, MF, P).transpose(2, 1, 0, 3)).astype(BF16NP)
        w3tt = np.ascontiguousarray(
            w3e.reshape(KH, P, MF, P).transpose(2, 1, 0, 3)).astype(BF16NP)
        w2tt = np.ascontiguousarray(
            w2e.reshape(KF, P, NW2, WG2, P).transpose(2, 1, 3, 0, 4)
        ).astype(BF16NP)
        in_maps.append({"xt": xt, "xts": xts,
                        "w1t": w1tt, "w3t": w3tt, "w2t": w2tt})
    return in_maps


def kernel(hidden_states, gate_w, w1, w2, w3):
    x = np.asarray(hidden_states, dtype=np.float32)
    gw = np.asarray(gate_w, dtype=np.float32)
    toks, cws, C = _plan(x, gw)
    if _NC_CACHE.get("C") != C:
        _NC_CACHE["nc"] = build(C)
        _NC_CACHE["C"] = C
    nc = _NC_CACHE["nc"]

    in_maps = make_in_maps(x, gw, w1, w2, w3)
    res = run_bass_kernel_spmd(nc, in_maps, core_ids=list(range(E)),
                               trace=False)
    out = np.zeros((T, H), dtype=np.float32)
    for e in range(E):
        tok = toks[e]
        if len(tok):
            out[tok] += res.results[e]["outT"][:, :len(tok)]\
                .astype(np.float32).T
    return out


# revision 12
# speedup vs baseline: 1.1434x; 1.0529x over previous
"""Mixtral-style MoE block (T=2048, H=1024, F=2048, E=8, top-2) on 8 trn2
NeuronCores.

Expert-parallel with host-side top-2 dispatch: the (tiny) router runs on the
host in fp32, exactly mirroring the reference math; each core receives only
the tokens routed to its expert (capacity C = max expert load), in bf16,
plus a second copy of the tokens pre-scaled by the renormalized top-2
combine weight (so the combine scaling rides the linear w3 branch for free).
Each core computes out_e = (silu(x w1) * (x_cw w3)) @ w2 in bf16 with fp32
PSUM accumulation and writes its [H, C] partial in bf16; the host
scatter-adds the two partials per token.  No device collectives; weights
stream once (bf16, 12.6 MB/core, hardware-DGE contiguous chunks) and the
kernel is PE-bound at ~78 TF/s bf16.
"""
import numpy as np

try:
    import concourse  # noqa: F401
except ImportError:  # pragma: no cover
    import sys
    sys.path.insert(0, "/opt/trn_rl_repo")

import ml_dtypes
from concourse import mybir, bacc
import concourse.tile as tile
from concourse.bass_utils import run_bass_kernel_spmd

T, H, F, E, TOP_K = 2048, 1024, 2048, 8, 2
P = 128
KH = H // P    # 8  k-tiles over H (mm1/mm3 contraction)
KF = F // P    # 16 k-tiles over F (mm2 contraction)
MF = F // P    # 16 m-tiles over F (mm1/mm3 output partitions)
MH = H // P    # 8  m-tiles over H (mm2 output partitions)
WG2 = 2        # m-tiles per w2 DMA chunk (1 MB each)
NW2 = MH // WG2
F32 = mybir.dt.float32
BF16 = mybir.dt.bfloat16
BF16NP = ml_dtypes.bfloat16
PSUM = "PSUM"

_NC_CACHE = {}


def _chunks(C):
    """Equal token-stream chunks of <=512 columns (PSUM bank limit)."""
    n = (C + 511) // 512
    base = C // n // 4 * 4
    cs, s = [], 0
    for i in range(n):
        e = C if i == n - 1 else s + base
        cs.append((s, e))
        s = e
    return cs


def build(C):
    nc = bacc.Bacc("TRN2", target_bir_lowering=False, debug=False,
                   num_devices=E)
    # host pre-tiles everything so each DMA is contiguous per partition
    xt = nc.dram_tensor("xt", [P, KH, C], BF16, kind="ExternalInput")
    xts = nc.dram_tensor("xts", [P, KH, C], BF16, kind="ExternalInput")
    w1t = nc.dram_tensor("w1t", [P, MF, KH, P], BF16, kind="ExternalInput")
    w3t = nc.dram_tensor("w3t", [P, MF, KH, P], BF16, kind="ExternalInput")
    w2t = nc.dram_tensor("w2t", [P, MH, KF, P], BF16, kind="ExternalInput")
    outT = nc.dram_tensor("outT", [H, C], BF16, kind="ExternalOutput")

    CH = _chunks(C)
    psbufs_a = 2 if len(CH) <= 2 else 1
    psbufs_b = 4 if len(CH) <= 2 else 2
    with tile.TileContext(nc) as tc:
        with (
            tc.tile_pool(name="big", bufs=1) as big,
            tc.tile_pool(name="evac", bufs=4) as evac,
        ):
            # PE warmup so HAM un-throttles before real work arrives
            jt = big.tile([P, 512], BF16)
            nc.gpsimd.memset(jt[:], 0.0)
            with tc.tile_pool(name="warm", bufs=1, space=PSUM) as wps:
                jp = wps.tile([P, 512], F32)
                for _ in range(10):
                    nc.tensor.matmul(jp[:], lhsT=jt[:, :P], rhs=jt[:],
                                     start=True, stop=True)

            # tokens in halves on the scalar HWDGE ring
            HK = KH // 2
            xt_h, xts_h = [], []
            for h in range(2):
                t = big.tile([P, HK, C], BF16, name=f"xt{h}")
                nc.scalar.dma_start(out=t[:],
                                    in_=xt.ap()[:, h * HK:(h + 1) * HK])
                xt_h.append(t)
            for h in range(2):
                t = big.tile([P, HK, C], BF16, name=f"xts{h}")
                nc.scalar.dma_start(out=t[:],
                                    in_=xts.ap()[:, h * HK:(h + 1) * HK])
                xts_h.append(t)

            def xtv(k):
                return xt_h[k // HK][:, k % HK]

            def xtsv(k):
                return xts_h[k // HK][:, k % HK]

            inter = big.tile([P, KF, C], BF16)

            # all weights SBUF-resident, contiguous DMAs on the sync
            # HWDGE ring, chunk sizes ramp up so the first m-tiles land fast
            WCH = [1, 1, 2, 4, 4, 4]
            w1c, w3c = [None] * MF, [None] * MF
            m0 = 0
            for g in WCH:
                t1 = big.tile([P, g, KH, P], BF16, name=f"w1c{m0}")
                nc.sync.dma_start(out=t1[:], in_=w1t.ap()[:, m0:m0 + g])
                t3 = big.tile([P, g, KH, P], BF16, name=f"w3c{m0}")
                nc.sync.dma_start(out=t3[:], in_=w3t.ap()[:, m0:m0 + g])
                for j in range(g):
                    w1c[m0 + j] = t1[:, j]
                    w3c[m0 + j] = t3[:, j]
                m0 += g
            w2c = [None] * MH
            for j in range(2):
                g = MH // 2
                t = big.tile([P, g, KF, P], BF16, name=f"w2c{j}")
                nc.sync.dma_start(out=t[:], in_=w2t.ap()[:, j * g:(j + 1) * g])
                for i in range(g):
                    w2c[j * g + i] = t[:, i]

            # Phase A: interT[f, t] = silu(w1.T x) * (w3.T x_cw)
            with tc.tile_pool(name="psA", bufs=psbufs_a, space=PSUM) as psA:
                for m in range(MF):
                    ps1 = [psA.tile([P, e - s], F32, tag=f"ps1_{i}",
                                    name=f"ps1_{i}")
                           for i, (s, e) in enumerate(CH)]
                    ps3 = [psA.tile([P, e - s], F32, tag=f"ps3_{i}",
                                    name=f"ps3_{i}")
                           for i, (s, e) in enumerate(CH)]
                    for i, (s, e) in enumerate(CH):
                        for k in range(KH):
                            nc.tensor.matmul(ps1[i][:], lhsT=w1c[m][:, k, :],
                                             rhs=xtv(k)[:, s:e],
                                             start=(k == 0), stop=(k == KH - 1))
                    for i, (s, e) in enumerate(CH):
                        for k in range(KH):
                            nc.tensor.matmul(ps3[i][:], lhsT=w3c[m][:, k, :],
                                             rhs=xtsv(k)[:, s:e],
                                             start=(k == 0), stop=(k == KH - 1))
                    for i, (s, e) in enumerate(CH):
                        sil = evac.tile([P, e - s], F32, tag=f"sil_{i}")
                        nc.scalar.activation(sil[:], ps1[i][:],
                                             mybir.ActivationFunctionType.Silu)
                        nc.vector.tensor_tensor(inter[:, m, s:e], sil[:],
                                                ps3[i][:],
                                                op=mybir.AluOpType.mult)

            # Phase B: outT[h, t] = w2.T @ inter
            with tc.tile_pool(name="psB", bufs=psbufs_b, space=PSUM) as psB:
                for m in range(MH):
                    w2m = w2c[m]
                    o = evac.tile([P, C], BF16, tag="o")
                    for i, (s, e) in enumerate(CH):
                        ps = psB.tile([P, e - s], F32, tag=f"ps_{i}",
                                      name=f"ps_{i}")
                        for k in range(KF):
                            nc.tensor.matmul(ps[:], lhsT=w2m[:, k, :],
                                             rhs=inter[:, k, s:e],
                                             start=(k == 0), stop=(k == KF - 1))
                        nc.vector.tensor_copy(o[:, s:e], ps[:])
                    eng = nc.sync if m % 2 == 0 else nc.scalar
                    eng.dma_start(out=outT.ap()[m * P:(m + 1) * P, :],
                                  in_=o[:])
    nc.compile()
    return nc


def _route(hidden_states, gate_w):
    """Host router mirroring the reference fp32 math exactly."""
    logits = (hidden_states.astype(np.float32) @
              gate_w.astype(np.float32)).astype(np.float32)
    mx = logits.max(axis=-1, keepdims=True)
    p = np.exp(logits - mx)
    p /= p.sum(axis=-1, keepdims=True)
    idx = np.argsort(-p, axis=-1, kind="stable")[:, :TOP_K]
    tw = np.take_along_axis(p, idx, axis=-1)
    tw = tw / tw.sum(axis=-1, keepdims=True)
    return idx, tw


def _plan(hidden_states, gate_w):
    idx, tw = _route(hidden_states, gate_w)
    toks, cws = [], []
    for e in range(E):
        mask = idx == e
        tok = np.nonzero(mask.any(axis=1))[0]
        toks.append(tok)
        cws.append((tw * mask).sum(axis=1)[tok].astype(np.float32))
    cap = max(1, max(len(t) for t in toks))
    C = (cap + 7) // 8 * 8
    return toks, cws, C


def make_in_maps(hidden_states, gate_w, w1, w2, w3):
    x = np.asarray(hidden_states, dtype=np.float32)
    toks, cws, C = _plan(x, np.asarray(gate_w, dtype=np.float32))
    in_maps = []
    for e in range(E):
        tok, cw = toks[e], cws[e]
        n = len(tok)
        xe = x[tok]                                   # [n, H] fp32
        xt = np.zeros((P, KH, C), dtype=BF16NP)
        xt[:, :, :n] = xe.T.reshape(KH, P, n).transpose(1, 0, 2).astype(BF16NP)
        xts = np.zeros((P, KH, C), dtype=BF16NP)
        xts[:, :, :n] = (xe * cw[:, None]).T.reshape(KH, P, n)\
            .transpose(1, 0, 2).astype(BF16NP)
        w1e = np.asarray(w1[e], dtype=np.float32)
        w3e = np.asarray(w3[e], dtype=np.float32)
        w2e = np.asarray(w2[e], dtype=np.float32)
        # [H, F] -> [P, MF, KH, P] so any m-range DMA is contiguous
        w1tt = np.ascontiguousarray(
            w1e.reshape(KH, P, MF, P).transpose(1, 2, 0, 3)).astype(BF16NP)
        w3tt = np.ascontiguousarray(
            w3e.reshape(KH, P, MF, P).transpose(1, 2, 0, 3)).astype(BF16NP)
        w2tt = np.ascontiguousarray(
            w2e.reshape(KF, P, MH, P).transpose(1, 2, 0, 3)).astype(BF16NP)
        in_maps.append({"xt": xt, "xts": xts,
                        "w1t": w1tt, "w3t": w3tt, "w2t": w2tt})
    return in_maps


def kernel(hidden_states, gate_w, w1, w2, w3):
    x = np.asarray(hidden_states, dtype=np.float32)
    gw = np.asarray(gate_w, dtype=np.float32)
    toks, cws, C = _plan(x, gw)
    if _NC_CACHE.get("C") != C:
        _NC_CACHE["nc"] = build(C)
        _NC_CACHE["C"] = C
    nc = _NC_CACHE["nc"]

    in_maps = make_in_maps(x, gw, w1, w2, w3)
    res = run_bass_kernel_spmd(nc, in_maps, core_ids=list(range(E)),
                               trace=False)
    out = np.zeros((T, H), dtype=np.float32)
    for e in range(E):
        tok = toks[e]
        if len(tok):
            out[tok] += res.results[e]["outT"][:, :len(tok)]\
                .astype(np.float32).T
    return out
